# revision 9
# baseline (speedup 1.0000x reference)
"""AttentionConv3D Trainium2 kernel.

Computation (per channel c, voxel (d,h,w)):
    q,k,v = 1x1x1 convs of x;  s_kv = q * (k_pad[nbr kv] + rel_bias(c,kv))
    out   = sum_kv softmax_kv(s) * v_pad[nbr kv]         (27 = 3x3x3 window)

Strategy: depth-shard over 8 cores (2 output depth planes each, 1-plane halo).
Host zero-pads x to [64,18,66,66] so the channel-mix matmuls directly produce
zero-padded k/v/q planes. On-device layout: partition p = dl*64 + c
(dl in {0,1} local depth), free dim = padded 66x66 plane (4356).
Per kv-neighbor the window access is a free-dim offset (kh-1)*66 + (kw-1) into
one of three depth-plane buffers K[kd]; the rel bias collapses to a
per-partition scalar B[p, kv], so s = (K_shift + B)*q is ONE DVE
scalar_tensor_tensor op. exp on ACT; numerator/denominator accumulated with
identity matmuls into PSUM on the TensorEngine; 1/den via exp(-ln(den)) on ACT.
"""

import sys
import numpy as np

for _p in ("/opt/trn_rl_repo", "/root/.axon_site/_ro/trn_rl_repo"):
    if _p not in sys.path:
        sys.path.insert(0, _p)

HP = 66               # padded plane edge
HW = HP * HP          # 4356
NPL = 4               # k/v depth planes per core (2 outputs + halo)
R0 = 67               # first interior padded-linear position
CHUNKS = [(67, 1408), (1475, 1408), (2883, 1406)]  # covers [67, 4289)
PROJ = [(0, 1536), (1536, 1536), (3072, 1284)]     # proj psum chunks over 4356

# hot-path dtype knobs (fp32 = safe; bf16 halves DVE cost of the e*v path)
E_BF16 = True   # e / v / ev tiles + identity in bf16 (PE still accums fp32)

_CACHE = {}


def _subs(L):
    return [(0, 512), (512, 512), (1024, L - 1024)]


def _build():
    from contextlib import ExitStack
    import concourse.bacc as bacc
    import concourse.tile as tile
    from concourse import mybir

    f32 = mybir.dt.float32
    bf16 = mybir.dt.bfloat16
    edt = bf16 if E_BF16 else f32
    Alu = mybir.AluOpType
    Act = mybir.ActivationFunctionType

    nc = bacc.Bacc("TRN2", target_bir_lowering=False)
    xs_d = nc.dram_tensor("xs", [64, NPL * HW], f32, kind="ExternalInput")
    wk_d = nc.dram_tensor("wk2", [64, 128], f32, kind="ExternalInput")
    wv_d = nc.dram_tensor("wv2", [64, 128], f32, kind="ExternalInput")
    wq_d = nc.dram_tensor("wq2", [64, 128], f32, kind="ExternalInput")
    b_d = nc.dram_tensor("bias", [128, 27], f32, kind="ExternalInput")
    id_d = nc.dram_tensor("ident", [128, 128], edt, kind="ExternalInput")
    out_d = nc.dram_tensor("out", [128, 64, 64], f32, kind="ExternalOutput")

    with tile.TileContext(nc) as tc, ExitStack() as ctx:
        singles = ctx.enter_context(tc.tile_pool(name="singles", bufs=1))
        planes = ctx.enter_context(tc.tile_pool(name="planes", bufs=1))
        wpool = ctx.enter_context(tc.tile_pool(name="work", bufs=2))

        wk_s = singles.tile([64, 128], f32, tag="wk")
        wv_s = singles.tile([64, 128], f32, tag="wv")
        wq_s = singles.tile([64, 128], f32, tag="wq")
        id_s = singles.tile([128, 128], edt, tag="id")
        b_s = singles.tile([128, 27], f32, tag="b")
        ebias = singles.tile([128, 1], f32, tag="ebias")
        nc.vector.memset(ebias[:], -28.0)
        for t, d in ((wk_s, wk_d), (wv_s, wv_d), (wq_s, wq_d),
                     (id_s, id_d), (b_s, b_d)):
            nc.sync.dma_start(t[:], d[:])

        Kp = [planes.tile([128, HW], f32, tag=f"k{i}", name=f"k{i}") for i in range(3)]
        Vp = [planes.tile([128, HW], edt, tag=f"v{i}", name=f"v{i}") for i in range(3)]
        Q = planes.tile([128, HW], f32, tag="q")
        OUT = planes.tile([128, HW], f32, tag="o")

        # ---- projections: plane m of xs -> k/v (dual-copy weights give the
        # same output plane on partitions 0:64 and 64:128), q for m in {1,2}.
        with tc.tile_pool(name="xp", bufs=2) as xpool, \
             tc.tile_pool(name="pp", bufs=2, space="PSUM") as ppool:
            for m in range(NPL):
                X = xpool.tile([64, HW], f32, tag="x")
                nc.sync.dma_start(X[:], xs_d[:, m * HW:(m + 1) * HW])
                projs = [(wk_s, "k"), (wv_s, "v")]
                if m in (1, 2):
                    projs.append((wq_s, "q"))
                for w_s, kind in projs:
                    for base, L3 in PROJ:
                        pp = ppool.tile([128, 1536], f32, tag="pp")
                        for a, bl in _subs(L3):
                            nc.tensor.matmul(pp[:, a:a + bl], w_s[:],
                                             X[:, base + a:base + a + bl],
                                             start=True, stop=True)
                        sl = (slice(0, 64), slice(base, base + L3))
                        sh = (slice(64, 128), slice(base, base + L3))
                        if kind == "k":
                            if m <= 2:
                                nc.vector.tensor_copy(Kp[m][sl], pp[0:64, :L3])
                            if m >= 1:
                                nc.vector.tensor_copy(Kp[m - 1][sh], pp[64:128, :L3])
                        elif kind == "v":
                            if m <= 2:
                                nc.scalar.copy(Vp[m][sl], pp[0:64, :L3])
                            if m >= 1:
                                nc.scalar.copy(Vp[m - 1][sh], pp[64:128, :L3])
                        elif m == 1:
                            nc.vector.tensor_copy(Q[sl], pp[0:64, :L3])
                        else:
                            nc.scalar.copy(Q[sh], pp[64:128, :L3])

        # ---- 27-neighbor softmax attention, PSUM-chunked over the plane
        accp = ctx.enter_context(tc.tile_pool(name="acc", bufs=1, space="PSUM"))
        for c0, L in CHUNKS:
            den = accp.tile([128, 1536], f32, tag="den")
            num = accp.tile([128, 1536], f32, tag="num")
            for kv in range(27):
                kd, r = divmod(kv, 9)
                kh, kw = divmod(r, 3)
                dd = (kh - 1) * HP + (kw - 1)
                s_t = wpool.tile([128, 1536], f32, tag="s")
                nc.vector.scalar_tensor_tensor(
                    s_t[:, :L], Kp[kd][:, c0 + dd:c0 + dd + L],
                    b_s[:, kv:kv + 1], Q[:, c0:c0 + L], Alu.add, Alu.mult)
                e_t = wpool.tile([128, 1536], edt, tag="e")
                # bias keeps exp inside the ACT table range (softmax is
                # shift-invariant; the -28 cancels via the ln/exp normalize)
                nc.scalar.activation(e_t[:, :L], s_t[:, :L], Act.Exp, bias=ebias[:])
                ev_t = wpool.tile([128, 1536], edt, tag="ev")
                # DVE is the bottleneck engine; hand ~half the e*v products
                # to the otherwise-idle GPSIMD (stock Q7 tensor_tensor).
                ev_eng = nc.gpsimd if (kw == 1 or kv in (0, 2, 18, 20)) else nc.vector
                ev_eng.tensor_mul(ev_t[:, :L], e_t[:, :L],
                                  Vp[kd][:, c0 + dd:c0 + dd + L])
                st, sp = kv == 0, kv == 26
                for a, bl in _subs(L):
                    nc.tensor.matmul(den[:, a:a + bl], id_s[:], e_t[:, a:a + bl],
                                     start=st, stop=sp)
                    nc.tensor.matmul(num[:, a:a + bl], id_s[:], ev_t[:, a:a + bl],
                                     start=st, stop=sp)
            l_t = wpool.tile([128, 1536], f32, tag="s")
            nc.scalar.activation(l_t[:, :L], den[:, :L], Act.Ln)
            f_t = wpool.tile([128, 1536], f32, tag="f")
            nc.scalar.activation(f_t[:, :L], l_t[:, :L], Act.Exp, scale=-1.0)
            nc.vector.tensor_mul(OUT[:, c0:c0 + L], num[:, :L], f_t[:, :L])

        OUTv = OUT.rearrange("p (r c) -> p r c", c=HP)
        nc.sync.dma_start(out_d[:, :, :], OUTv[:, 1:65, 1:65])
    nc.finalize()
    return nc


def kernel(x, w_q, w_k, w_v, rel_d, rel_h, rel_w):
    from concourse.bass_utils import run_bass_kernel_spmd

    x = np.asarray(x, np.float32)
    rd = np.asarray(rel_d, np.float32).reshape(21, 3)
    rh = np.asarray(rel_h, np.float32).reshape(21, 3)
    rw = np.asarray(rel_w, np.float32).reshape(22, 3)

    xp = np.zeros((64, 18, HP, HP), np.float32)
    xp[:, 1:17, 1:65, 1:65] = x[0]

    B = np.zeros((128, 27), np.float32)
    for c in range(64):
        for kv in range(27):
            kd, r = divmod(kv, 9)
            kh, kw = divmod(r, 3)
            b = rd[c, kd] if c < 21 else (rh[c - 21, kh] if c < 42 else rw[c - 42, kw])
            B[c, kv] = B[64 + c, kv] = b

    idt = np.eye(128, dtype=np.float32)
    if E_BF16:
        import ml_dtypes
        idt = idt.astype(ml_dtypes.bfloat16)
    com = {
        "wk2": np.concatenate([w_k.T, w_k.T], 1).astype(np.float32).copy(),
        "wv2": np.concatenate([w_v.T, w_v.T], 1).astype(np.float32).copy(),
        "wq2": np.concatenate([w_q.T, w_q.T], 1).astype(np.float32).copy(),
        "bias": B, "ident": idt,
    }
    in_maps = []
    for i in range(8):
        m = dict(com)
        m["xs"] = xp[:, 2 * i:2 * i + 4].reshape(64, NPL * HW).copy()
        in_maps.append(m)

    if "nc" not in _CACHE:
        _CACHE["nc"] = _build()
    res = run_bass_kernel_spmd(_CACHE["nc"], in_maps, list(range(8)))

    out = np.empty((1, 64, 16, 64, 64), np.float32)
    for i in range(8):
        arr = res.results[i]["out"].reshape(2, 64, 64, 64)
        out[0, :, 2 * i] = arr[0]
        out[0, :, 2 * i + 1] = arr[1]
    return out


# revision 16
# speedup vs baseline: 1.0953x; 1.0953x over previous
"""AttentionConv3D Trainium2 kernel.

Computation (per channel c, voxel (d,h,w)):
    q,k,v = 1x1x1 convs of x;  s_kv = q * (k_pad[nbr kv] + rel_bias(c,kv))
    out   = sum_kv softmax_kv(s) * v_pad[nbr kv]         (27 = 3x3x3 window)

Strategy: depth-shard over 8 cores (2 output depth planes each, 1-plane halo).
Host zero-pads x to [64,18,66,66] so the channel-mix matmuls directly produce
zero-padded k/v/q planes. On-device layout: partition p = dl*64 + c
(dl in {0,1} local depth), free dim = padded 66x66 plane (4356).
Per kv-neighbor the window access is a free-dim offset (kh-1)*66 + (kw-1) into
one of three depth-plane buffers K[kd]; the rel bias collapses to a
per-partition scalar B[p, kv], so s = (K_shift + B)*q is ONE DVE
scalar_tensor_tensor op. exp on ACT; numerator/denominator accumulated with
identity matmuls into PSUM on the TensorEngine; 1/den via exp(-ln(den)) on ACT.
"""

import sys
import numpy as np

for _p in ("/opt/trn_rl_repo", "/root/.axon_site/_ro/trn_rl_repo"):
    if _p not in sys.path:
        sys.path.insert(0, _p)

HP = 66               # padded plane edge
HW = HP * HP          # 4356
NPL = 4               # k/v depth planes per core (2 outputs + halo)
R0 = 67               # first interior padded-linear position
CHUNKS = [(67, 1402), (1469, 1402), (2871, 1418)]  # covers [67, 4289); chunk 0's
# window reads ([67-67, 67+67+1402) = [0,1536)) fit inside proj col-chunk 0 so
# the kv loop overlaps the tail of the projection phase.
PROJ = [(0, 1536), (1536, 1536), (3072, 1284)]     # proj psum chunks over 4356
OUT_ROWS = [(0, 21), (21, 42), (42, 64)]           # row bands DMA'd per chunk

# hot-path dtype knobs (fp32 = safe; bf16 halves DVE cost of the e*v path)
E_BF16 = True   # e / v / ev tiles + identity in bf16 (PE still accums fp32)

_CACHE = {}


def _subs(L):
    return [(0, 512), (512, 512), (1024, L - 1024)]


def _build():
    from contextlib import ExitStack
    import concourse.bacc as bacc
    import concourse.tile as tile
    from concourse import mybir

    f32 = mybir.dt.float32
    bf16 = mybir.dt.bfloat16
    edt = bf16 if E_BF16 else f32
    Alu = mybir.AluOpType
    Act = mybir.ActivationFunctionType

    nc = bacc.Bacc("TRN2", target_bir_lowering=False)
    xs_d = nc.dram_tensor("xs", [64, NPL * HW], f32, kind="ExternalInput")
    wk_d = nc.dram_tensor("wk2", [64, 128], f32, kind="ExternalInput")
    wv_d = nc.dram_tensor("wv2", [64, 128], f32, kind="ExternalInput")
    wq_d = nc.dram_tensor("wq2", [64, 128], f32, kind="ExternalInput")
    b_d = nc.dram_tensor("bias", [128, 27], f32, kind="ExternalInput")
    id_d = nc.dram_tensor("ident", [128, 128], edt, kind="ExternalInput")
    out_d = nc.dram_tensor("out", [128, 64, 64], f32, kind="ExternalOutput")

    with tile.TileContext(nc) as tc, ExitStack() as ctx:
        singles = ctx.enter_context(tc.tile_pool(name="singles", bufs=1))
        planes = ctx.enter_context(tc.tile_pool(name="planes", bufs=1))
        wpool = ctx.enter_context(tc.tile_pool(name="work", bufs=2))

        wk_s = singles.tile([64, 128], f32, tag="wk")
        wv_s = singles.tile([64, 128], f32, tag="wv")
        wq_s = singles.tile([64, 128], f32, tag="wq")
        id_s = singles.tile([128, 128], edt, tag="id")
        b_s = singles.tile([128, 27], f32, tag="b")
        ebias = singles.tile([128, 1], f32, tag="ebias")
        nc.vector.memset(ebias[:], -28.0)
        for t, d in ((wk_s, wk_d), (wv_s, wv_d), (wq_s, wq_d),
                     (id_s, id_d), (b_s, b_d)):
            nc.sync.dma_start(t[:], d[:])

        Kp = [planes.tile([128, HW], f32, tag=f"k{i}", name=f"k{i}") for i in range(3)]
        Vp = [planes.tile([128, HW], edt, tag=f"v{i}", name=f"v{i}") for i in range(3)]
        Q = planes.tile([128, HW], f32, tag="q")
        OUT = planes.tile([128, HW], f32, tag="o")

        # ---- projections: plane m of xs -> k/v (dual-copy weights give the
        # same output plane on partitions 0:64 and 64:128), q for m in {1,2}.
        # column-chunk OUTER so all planes' first 1536 columns (what kv chunk 0
        # needs) are projected before any plane's later columns.
        with tc.tile_pool(name="xp", bufs=3) as xpool, \
             tc.tile_pool(name="pp", bufs=2, space="PSUM") as ppool:
            for base, L3 in PROJ:
                for m in range(NPL):
                    X = xpool.tile([64, 1536], f32, tag="x")
                    nc.sync.dma_start(X[:, :L3],
                                      xs_d[:, m * HW + base:m * HW + base + L3])
                    projs = [(wk_s, "k"), (wv_s, "v")]
                    if m in (1, 2):
                        projs.append((wq_s, "q"))
                    for w_s, kind in projs:
                        pp = ppool.tile([128, 1536], f32, tag="pp")
                        for a, bl in _subs(L3):
                            nc.tensor.matmul(pp[:, a:a + bl], w_s[:],
                                             X[:, a:a + bl],
                                             start=True, stop=True)
                        sl = (slice(0, 64), slice(base, base + L3))
                        sh = (slice(64, 128), slice(base, base + L3))
                        if kind == "k":
                            if m <= 2:
                                nc.vector.tensor_copy(Kp[m][sl], pp[0:64, :L3])
                            if m >= 1:
                                nc.vector.tensor_copy(Kp[m - 1][sh], pp[64:128, :L3])
                        elif kind == "v":
                            if m <= 2:
                                nc.scalar.copy(Vp[m][sl], pp[0:64, :L3])
                            if m >= 1:
                                nc.scalar.copy(Vp[m - 1][sh], pp[64:128, :L3])
                        elif m == 1:
                            nc.vector.tensor_copy(Q[sl], pp[0:64, :L3])
                        else:
                            nc.scalar.copy(Q[sh], pp[64:128, :L3])

        # ---- 27-neighbor softmax attention, PSUM-chunked over the plane
        accp = ctx.enter_context(tc.tile_pool(name="acc", bufs=1, space="PSUM"))
        OUTv = OUT.rearrange("p (r c) -> p r c", c=HP)
        GPSET = frozenset((0, 2, 6, 9, 11, 15, 17, 18, 20, 24, 26))
        for (c0, L), (r0, r1) in zip(CHUNKS, OUT_ROWS):
            den = accp.tile([128, 1536], f32, tag="den")
            num = accp.tile([128, 1536], f32, tag="num")
            for kv in range(27):
                kd, r = divmod(kv, 9)
                kh, kw = divmod(r, 3)
                dd = (kh - 1) * HP + (kw - 1)
                s_t = wpool.tile([128, 1536], f32, tag="s")
                nc.vector.scalar_tensor_tensor(
                    s_t[:, :L], Kp[kd][:, c0 + dd:c0 + dd + L],
                    b_s[:, kv:kv + 1], Q[:, c0:c0 + L], Alu.add, Alu.mult)
                e_t = wpool.tile([128, 1536], edt, tag="e")
                # bias keeps exp inside the ACT table range (softmax is
                # shift-invariant; the -28 cancels via the ln/exp normalize)
                nc.scalar.activation(e_t[:, :L], s_t[:, :L], Act.Exp, bias=ebias[:])
                ev_t = wpool.tile([128, 1536], edt, tag="ev")
                # DVE is the bottleneck engine; hand ~half the e*v products
                # to the otherwise-idle GPSIMD (stock Q7 tensor_tensor).
                ev_eng = nc.gpsimd if (kw == 1 or kv in GPSET) else nc.vector
                ev_eng.tensor_mul(ev_t[:, :L], e_t[:, :L],
                                  Vp[kd][:, c0 + dd:c0 + dd + L])
                st, sp = kv == 0, kv == 26
                for a, bl in _subs(L):
                    nc.tensor.matmul(den[:, a:a + bl], id_s[:], e_t[:, a:a + bl],
                                     start=st, stop=sp)
                    nc.tensor.matmul(num[:, a:a + bl], id_s[:], ev_t[:, a:a + bl],
                                     start=st, stop=sp)
            l_t = wpool.tile([128, 1536], f32, tag="s")
            nc.scalar.activation(l_t[:, :L], den[:, :L], Act.Ln)
            f_t = wpool.tile([128, 1536], f32, tag="f")
            nc.scalar.activation(f_t[:, :L], l_t[:, :L], Act.Exp, scale=-1.0)
            nc.vector.tensor_mul(OUT[:, c0:c0 + L], num[:, :L], f_t[:, :L])
            # rows fully covered by chunks <= this one stream out immediately
            nc.sync.dma_start(out_d[:, r0:r1, :],
                              OUTv[:, 1 + r0:1 + r1, 1:65])
    nc.finalize()
    return nc


def kernel(x, w_q, w_k, w_v, rel_d, rel_h, rel_w):
    from concourse.bass_utils import run_bass_kernel_spmd

    x = np.asarray(x, np.float32)
    rd = np.asarray(rel_d, np.float32).reshape(21, 3)
    rh = np.asarray(rel_h, np.float32).reshape(21, 3)
    rw = np.asarray(rel_w, np.float32).reshape(22, 3)

    xp = np.zeros((64, 18, HP, HP), np.float32)
    xp[:, 1:17, 1:65, 1:65] = x[0]

    B = np.zeros((128, 27), np.float32)
    for c in range(64):
        for kv in range(27):
            kd, r = divmod(kv, 9)
            kh, kw = divmod(r, 3)
            b = rd[c, kd] if c < 21 else (rh[c - 21, kh] if c < 42 else rw[c - 42, kw])
            B[c, kv] = B[64 + c, kv] = b

    idt = np.eye(128, dtype=np.float32)
    if E_BF16:
        import ml_dtypes
        idt = idt.astype(ml_dtypes.bfloat16)
    com = {
        "wk2": np.concatenate([w_k.T, w_k.T], 1).astype(np.float32).copy(),
        "wv2": np.concatenate([w_v.T, w_v.T], 1).astype(np.float32).copy(),
        "wq2": np.concatenate([w_q.T, w_q.T], 1).astype(np.float32).copy(),
        "bias": B, "ident": idt,
    }
    in_maps = []
    for i in range(8):
        m = dict(com)
        m["xs"] = xp[:, 2 * i:2 * i + 4].reshape(64, NPL * HW).copy()
        in_maps.append(m)

    if "nc" not in _CACHE:
        _CACHE["nc"] = _build()
    res = run_bass_kernel_spmd(_CACHE["nc"], in_maps, list(range(8)))

    out = np.empty((1, 64, 16, 64, 64), np.float32)
    for i in range(8):
        arr = res.results[i]["out"].reshape(2, 64, 64, 64)
        out[0, :, 2 * i] = arr[0]
        out[0, :, 2 * i + 1] = arr[1]
    return out


# revision 18
# speedup vs baseline: 1.2022x; 1.0976x over previous
"""AttentionConv3D Trainium2 kernel.

Computation (per channel c, voxel (d,h,w)):
    q,k,v = 1x1x1 convs of x;  s_kv = q * (k_pad[nbr kv] + rel_bias(c,kv))
    out   = sum_kv softmax_kv(s) * v_pad[nbr kv]         (27 = 3x3x3 window)

Strategy: depth-shard over 8 cores (2 output depth planes each, 1-plane halo).
Host zero-pads x to [64,18,66,66] so the channel-mix matmuls directly produce
zero-padded k/v/q planes. On-device layout: partition p = dl*64 + c
(dl in {0,1} local depth), free dim = padded 66x66 plane (4356).
Per kv-neighbor the window access is a free-dim offset (kh-1)*66 + (kw-1) into
one of three depth-plane buffers K[kd]; the rel bias collapses to a
per-partition scalar B[p, kv], so s = (K_shift + B)*q is ONE DVE
scalar_tensor_tensor op. exp on ACT; numerator/denominator accumulated with
identity matmuls into PSUM on the TensorEngine; 1/den via exp(-ln(den)) on ACT.
"""

import sys
import numpy as np

for _p in ("/opt/trn_rl_repo", "/root/.axon_site/_ro/trn_rl_repo"):
    if _p not in sys.path:
        sys.path.insert(0, _p)

HP = 66               # padded plane edge
HW = HP * HP          # 4356
NPL = 4               # k/v depth planes per core (2 outputs + halo)
R0 = 67               # first interior padded-linear position
CHUNKS = [(67, 1402), (1469, 1402), (2871, 1418)]  # covers [67, 4289); chunk 0's
# window reads ([67-67, 67+67+1402) = [0,1536)) fit inside proj col-chunk 0 so
# the kv loop overlaps the tail of the projection phase.
PROJ = [(0, 1536), (1536, 1536), (3072, 1284)]     # proj psum chunks over 4356
OUT_ROWS = [(0, 21), (21, 42), (42, 64)]           # row bands DMA'd per chunk

# hot-path dtype knobs (fp32 = safe; bf16 halves DVE cost of the e*v path)
E_BF16 = True   # e / v / ev tiles + identity in bf16 (PE still accums fp32)

_CACHE = {}


def _subs(L):
    return [(0, 512), (512, 512), (1024, L - 1024)]


def _build():
    from contextlib import ExitStack
    import concourse.bacc as bacc
    import concourse.tile as tile
    from concourse import mybir

    f32 = mybir.dt.float32
    bf16 = mybir.dt.bfloat16
    edt = bf16 if E_BF16 else f32
    Alu = mybir.AluOpType
    Act = mybir.ActivationFunctionType

    nc = bacc.Bacc("TRN2", target_bir_lowering=False)
    xs_d = nc.dram_tensor("xs", [64, NPL * HW], f32, kind="ExternalInput")
    wk_d = nc.dram_tensor("wk2", [64, 128], f32, kind="ExternalInput")
    wv_d = nc.dram_tensor("wv2", [64, 128], f32, kind="ExternalInput")
    wq_d = nc.dram_tensor("wq2", [64, 128], f32, kind="ExternalInput")
    b_d = nc.dram_tensor("bias", [128, 27], f32, kind="ExternalInput")
    id_d = nc.dram_tensor("ident", [128, 128], edt, kind="ExternalInput")
    out_d = nc.dram_tensor("out", [128, 64, 64], f32, kind="ExternalOutput")

    with tile.TileContext(nc) as tc, ExitStack() as ctx:
        singles = ctx.enter_context(tc.tile_pool(name="singles", bufs=1))
        planes = ctx.enter_context(tc.tile_pool(name="planes", bufs=1))
        wpool = ctx.enter_context(tc.tile_pool(name="work", bufs=2))

        wk_s = singles.tile([64, 128], f32, tag="wk")
        wv_s = singles.tile([64, 128], f32, tag="wv")
        wq_s = singles.tile([64, 128], f32, tag="wq")
        id_s = singles.tile([128, 128], edt, tag="id")
        b_s = singles.tile([128, 27], f32, tag="b")
        ebias = singles.tile([128, 1], f32, tag="ebias")
        nc.vector.memset(ebias[:], -28.0)
        for t, d in ((wk_s, wk_d), (wv_s, wv_d), (wq_s, wq_d),
                     (id_s, id_d), (b_s, b_d)):
            nc.sync.dma_start(t[:], d[:])

        Kp = [planes.tile([128, HW], f32, tag=f"k{i}", name=f"k{i}") for i in range(3)]
        Vp = [planes.tile([128, HW], edt, tag=f"v{i}", name=f"v{i}") for i in range(3)]
        Q = planes.tile([128, HW], f32, tag="q")
        OUT = planes.tile([128, HW], f32, tag="o")

        # ---- projections: plane m of xs -> k/v (dual-copy weights give the
        # same output plane on partitions 0:64 and 64:128), q for m in {1,2}.
        # column-chunk OUTER so all planes' first 1536 columns (what kv chunk 0
        # needs) are projected before any plane's later columns.
        with tc.tile_pool(name="xp", bufs=3) as xpool, \
             tc.tile_pool(name="pp", bufs=2, space="PSUM") as ppool:
            for base, L3 in PROJ:
                for m in range(NPL):
                    X = xpool.tile([64, 1536], f32, tag="x")
                    nc.sync.dma_start(X[:, :L3],
                                      xs_d[:, m * HW + base:m * HW + base + L3])
                    projs = [(wk_s, "k"), (wv_s, "v")]
                    if m in (1, 2):
                        projs.append((wq_s, "q"))
                    for w_s, kind in projs:
                        pp = ppool.tile([128, 1536], f32, tag="pp")
                        for a, bl in _subs(L3):
                            nc.tensor.matmul(pp[:, a:a + bl], w_s[:],
                                             X[:, a:a + bl],
                                             start=True, stop=True)
                        sl = (slice(0, 64), slice(base, base + L3))
                        sh = (slice(64, 128), slice(base, base + L3))
                        if kind == "k":
                            # split k evacuations across DVE/ACT to keep DVE,
                            # the span-limiting engine, under ACT's load
                            if m <= 2:
                                nc.vector.tensor_copy(Kp[m][sl], pp[0:64, :L3])
                            if m >= 1:
                                nc.scalar.copy(Kp[m - 1][sh], pp[64:128, :L3])
                        elif kind == "v":
                            if m <= 2:
                                nc.scalar.copy(Vp[m][sl], pp[0:64, :L3])
                            if m >= 1:
                                nc.scalar.copy(Vp[m - 1][sh], pp[64:128, :L3])
                        elif m == 1:
                            nc.vector.tensor_copy(Q[sl], pp[0:64, :L3])
                        else:
                            nc.scalar.copy(Q[sh], pp[64:128, :L3])

        # ---- 27-neighbor softmax attention, PSUM-chunked over the plane
        accp = ctx.enter_context(tc.tile_pool(name="acc", bufs=1, space="PSUM"))
        OUTv = OUT.rearrange("p (r c) -> p r c", c=HP)
        GPSET = frozenset((0, 2, 6, 8, 9, 11, 15, 17, 18, 20, 21, 23, 24, 26))
        for (c0, L), (r0, r1) in zip(CHUNKS, OUT_ROWS):
            den = accp.tile([128, 1536], f32, tag="den")
            num = accp.tile([128, 1536], f32, tag="num")
            for kv in range(27):
                kd, r = divmod(kv, 9)
                kh, kw = divmod(r, 3)
                dd = (kh - 1) * HP + (kw - 1)
                s_t = wpool.tile([128, 1536], f32, tag="s")
                nc.vector.scalar_tensor_tensor(
                    s_t[:, :L], Kp[kd][:, c0 + dd:c0 + dd + L],
                    b_s[:, kv:kv + 1], Q[:, c0:c0 + L], Alu.add, Alu.mult)
                e_t = wpool.tile([128, 1536], edt, tag="e")
                # bias keeps exp inside the ACT table range (softmax is
                # shift-invariant; the -28 cancels via the ln/exp normalize)
                nc.scalar.activation(e_t[:, :L], s_t[:, :L], Act.Exp, bias=ebias[:])
                ev_t = wpool.tile([128, 1536], edt, tag="ev")
                # DVE is the bottleneck engine; hand ~half the e*v products
                # to the otherwise-idle GPSIMD (stock Q7 tensor_tensor).
                ev_eng = nc.gpsimd if (kw == 1 or kv in GPSET) else nc.vector
                ev_eng.tensor_mul(ev_t[:, :L], e_t[:, :L],
                                  Vp[kd][:, c0 + dd:c0 + dd + L])
                st, sp = kv == 0, kv == 26
                for a, bl in _subs(L):
                    nc.tensor.matmul(den[:, a:a + bl], id_s[:], e_t[:, a:a + bl],
                                     start=st, stop=sp)
                    nc.tensor.matmul(num[:, a:a + bl], id_s[:], ev_t[:, a:a + bl],
                                     start=st, stop=sp)
            l_t = wpool.tile([128, 1536], f32, tag="s")
            nc.scalar.activation(l_t[:, :L], den[:, :L], Act.Ln)
            f_t = wpool.tile([128, 1536], f32, tag="f")
            nc.scalar.activation(f_t[:, :L], l_t[:, :L], Act.Exp, scale=-1.0)
            nc.vector.tensor_mul(OUT[:, c0:c0 + L], num[:, :L], f_t[:, :L])
            # rows fully covered by chunks <= this one stream out immediately
            nc.sync.dma_start(out_d[:, r0:r1, :],
                              OUTv[:, 1 + r0:1 + r1, 1:65])
    nc.finalize()
    return nc


def kernel(x, w_q, w_k, w_v, rel_d, rel_h, rel_w):
    from concourse.bass_utils import run_bass_kernel_spmd

    x = np.asarray(x, np.float32)
    rd = np.asarray(rel_d, np.float32).reshape(21, 3)
    rh = np.asarray(rel_h, np.float32).reshape(21, 3)
    rw = np.asarray(rel_w, np.float32).reshape(22, 3)

    xp = np.zeros((64, 18, HP, HP), np.float32)
    xp[:, 1:17, 1:65, 1:65] = x[0]

    B = np.zeros((128, 27), np.float32)
    for c in range(64):
        for kv in range(27):
            kd, r = divmod(kv, 9)
            kh, kw = divmod(r, 3)
            b = rd[c, kd] if c < 21 else (rh[c - 21, kh] if c < 42 else rw[c - 42, kw])
            B[c, kv] = B[64 + c, kv] = b

    idt = np.eye(128, dtype=np.float32)
    if E_BF16:
        import ml_dtypes
        idt = idt.astype(ml_dtypes.bfloat16)
    com = {
        "wk2": np.concatenate([w_k.T, w_k.T], 1).astype(np.float32).copy(),
        "wv2": np.concatenate([w_v.T, w_v.T], 1).astype(np.float32).copy(),
        "wq2": np.concatenate([w_q.T, w_q.T], 1).astype(np.float32).copy(),
        "bias": B, "ident": idt,
    }
    in_maps = []
    for i in range(8):
        m = dict(com)
        m["xs"] = xp[:, 2 * i:2 * i + 4].reshape(64, NPL * HW).copy()
        in_maps.append(m)

    if "nc" not in _CACHE:
        _CACHE["nc"] = _build()
    res = run_bass_kernel_spmd(_CACHE["nc"], in_maps, list(range(8)))

    out = np.empty((1, 64, 16, 64, 64), np.float32)
    for i in range(8):
        arr = res.results[i]["out"].reshape(2, 64, 64, 64)
        out[0, :, 2 * i] = arr[0]
        out[0, :, 2 * i + 1] = arr[1]
    return out


# revision 22
# speedup vs baseline: 1.2073x; 1.0043x over previous
"""AttentionConv3D Trainium2 kernel.

Computation (per channel c, voxel (d,h,w)):
    q,k,v = 1x1x1 convs of x;  s_kv = q * (k_pad[nbr kv] + rel_bias(c,kv))
    out   = sum_kv softmax_kv(s) * v_pad[nbr kv]         (27 = 3x3x3 window)

Strategy: depth-shard over 8 cores (2 output depth planes each, 1-plane halo).
Host zero-pads x to [64,18,66,66] so the channel-mix matmuls directly produce
zero-padded k/v/q planes. On-device layout: partition p = dl*64 + c
(dl in {0,1} local depth), free dim = padded 66x66 plane (4356).
Per kv-neighbor the window access is a free-dim offset (kh-1)*66 + (kw-1) into
one of three depth-plane buffers K[kd]; the rel bias collapses to a
per-partition scalar B[p, kv], so s = (K_shift + B)*q is ONE DVE
scalar_tensor_tensor op. exp on ACT; numerator/denominator accumulated with
identity matmuls into PSUM on the TensorEngine; 1/den via exp(-ln(den)) on ACT.
"""

import sys
import numpy as np

for _p in ("/opt/trn_rl_repo", "/root/.axon_site/_ro/trn_rl_repo"):
    if _p not in sys.path:
        sys.path.insert(0, _p)

HP = 66               # padded plane edge
HW = HP * HP          # 4356
NPL = 4               # k/v depth planes per core (2 outputs + halo)
R0 = 67               # first interior padded-linear position
CHUNKS = [(67, 1402), (1469, 1402), (2871, 1418)]  # covers [67, 4289); chunk 0's
# window reads ([67-67, 67+67+1402) = [0,1536)) fit inside proj col-chunk 0 so
# the kv loop overlaps the tail of the projection phase.
PROJ = [(0, 1536), (1536, 1536), (3072, 1284)]     # proj psum chunks over 4356
OUT_ROWS = [(0, 21), (21, 42), (42, 64)]           # row bands DMA'd per chunk

# hot-path dtype knobs (fp32 = safe; bf16 halves DVE cost of the e*v path)
E_BF16 = True   # e / v / ev tiles + identity in bf16 (PE still accums fp32)

_CACHE = {}


def _subs(L):
    return [(0, 512), (512, 512), (1024, L - 1024)]


def _build():
    from contextlib import ExitStack
    import concourse.bacc as bacc
    import concourse.tile as tile
    from concourse import mybir

    f32 = mybir.dt.float32
    bf16 = mybir.dt.bfloat16
    edt = bf16 if E_BF16 else f32
    Alu = mybir.AluOpType
    Act = mybir.ActivationFunctionType

    nc = bacc.Bacc("TRN2", target_bir_lowering=False)
    xs_d = nc.dram_tensor("xs", [64, NPL * HW], f32, kind="ExternalInput")
    wk_d = nc.dram_tensor("wk2", [64, 128], f32, kind="ExternalInput")
    wv_d = nc.dram_tensor("wv2", [64, 128], f32, kind="ExternalInput")
    wq_d = nc.dram_tensor("wq2", [64, 128], f32, kind="ExternalInput")
    b_d = nc.dram_tensor("bias", [128, 27], f32, kind="ExternalInput")
    id_d = nc.dram_tensor("ident", [128, 128], edt, kind="ExternalInput")
    out_d = nc.dram_tensor("out", [128, 64, 64], f32, kind="ExternalOutput")

    with tile.TileContext(nc) as tc, ExitStack() as ctx:
        singles = ctx.enter_context(tc.tile_pool(name="singles", bufs=1))
        planes = ctx.enter_context(tc.tile_pool(name="planes", bufs=1))
        wpool = ctx.enter_context(tc.tile_pool(name="work", bufs=2))

        wk_s = singles.tile([64, 128], f32, tag="wk")
        wv_s = singles.tile([64, 128], f32, tag="wv")
        wq_s = singles.tile([64, 128], f32, tag="wq")
        id_s = singles.tile([128, 128], edt, tag="id")
        b_s = singles.tile([128, 27], f32, tag="b")
        ebias = singles.tile([128, 1], f32, tag="ebias")
        nc.vector.memset(ebias[:], -28.0)
        for t, d in ((wk_s, wk_d), (wv_s, wv_d), (wq_s, wq_d),
                     (id_s, id_d), (b_s, b_d)):
            nc.sync.dma_start(t[:], d[:])

        Kp = [planes.tile([128, HW], f32, tag=f"k{i}", name=f"k{i}") for i in range(3)]
        Vp = [planes.tile([128, HW], edt, tag=f"v{i}", name=f"v{i}") for i in range(3)]
        Q = planes.tile([128, HW], f32, tag="q")
        OUT = planes.tile([128, HW], f32, tag="o")

        # ---- projections: plane m of xs -> k/v (dual-copy weights give the
        # same output plane on partitions 0:64 and 64:128), q for m in {1,2}.
        # column-chunk OUTER so all planes' first 1536 columns (what kv chunk 0
        # needs) are projected before any plane's later columns.
        with tc.tile_pool(name="xp", bufs=3) as xpool, \
             tc.tile_pool(name="pp", bufs=2, space="PSUM") as ppool:
            for base, L3 in PROJ:
                for m in range(NPL):
                    X = xpool.tile([64, 1536], f32, tag="x")
                    nc.sync.dma_start(X[:, :L3],
                                      xs_d[:, m * HW + base:m * HW + base + L3])
                    projs = [(wk_s, "k"), (wv_s, "v")]
                    if m in (1, 2):
                        projs.append((wq_s, "q"))
                    for w_s, kind in projs:
                        pp = ppool.tile([128, 1536], f32, tag="pp")
                        for a, bl in _subs(L3):
                            nc.tensor.matmul(pp[:, a:a + bl], w_s[:],
                                             X[:, a:a + bl],
                                             start=True, stop=True)
                        sl = (slice(0, 64), slice(base, base + L3))
                        sh = (slice(64, 128), slice(base, base + L3))
                        if kind == "k":
                            # split k evacuations across DVE/ACT to keep DVE,
                            # the span-limiting engine, under ACT's load
                            if m <= 2:
                                nc.vector.tensor_copy(Kp[m][sl], pp[0:64, :L3])
                            if m >= 1:
                                nc.scalar.copy(Kp[m - 1][sh], pp[64:128, :L3])
                        elif kind == "v":
                            if m <= 2:
                                nc.scalar.copy(Vp[m][sl], pp[0:64, :L3])
                            if m >= 1:
                                nc.scalar.copy(Vp[m - 1][sh], pp[64:128, :L3])
                        elif m == 1:
                            nc.vector.tensor_copy(Q[sl], pp[0:64, :L3])
                        else:
                            nc.scalar.copy(Q[sh], pp[64:128, :L3])

        # ---- 27-neighbor softmax attention, PSUM-chunked over the plane
        accp = ctx.enter_context(tc.tile_pool(name="acc", bufs=1, space="PSUM"))
        OUTv = OUT.rearrange("p (r c) -> p r c", c=HP)
        GPSET = frozenset((0, 2, 6, 8, 9, 11, 15, 17, 18, 20, 21, 23, 24, 26))
        for (c0, L), (r0, r1) in zip(CHUNKS, OUT_ROWS):
            den = accp.tile([128, 1536], f32, tag="den")
            num = accp.tile([128, 1536], f32, tag="num")
            for kv in range(27):
                kd, r = divmod(kv, 9)
                kh, kw = divmod(r, 3)
                dd = (kh - 1) * HP + (kw - 1)
                s_t = wpool.tile([128, 1536], f32, tag="s")
                nc.vector.scalar_tensor_tensor(
                    s_t[:, :L], Kp[kd][:, c0 + dd:c0 + dd + L],
                    b_s[:, kv:kv + 1], Q[:, c0:c0 + L], Alu.add, Alu.mult)
                e_t = wpool.tile([128, 1536], edt, tag="e")
                # bias keeps exp inside the ACT table range (softmax is
                # shift-invariant; the -28 cancels via the ln/exp normalize)
                nc.scalar.activation(e_t[:, :L], s_t[:, :L], Act.Exp, bias=ebias[:])
                ev_t = wpool.tile([128, 1536], edt, tag="ev")
                # DVE is the bottleneck engine; hand ~half the e*v products
                # to the otherwise-idle GPSIMD (stock Q7 tensor_tensor).
                ev_eng = nc.gpsimd if (kw == 1 or kv in GPSET) else nc.vector
                ev_eng.tensor_mul(ev_t[:, :L], e_t[:, :L],
                                  Vp[kd][:, c0 + dd:c0 + dd + L])
                st, sp = kv == 0, kv == 26
                for a, bl in _subs(L):
                    nc.tensor.matmul(den[:, a:a + bl], id_s[:], e_t[:, a:a + bl],
                                     start=st, stop=sp)
                    nc.tensor.matmul(num[:, a:a + bl], id_s[:], ev_t[:, a:a + bl],
                                     start=st, stop=sp)
            l_t = wpool.tile([128, 1536], f32, tag="s")
            nc.scalar.activation(l_t[:, :L], den[:, :L], Act.Ln)
            f_t = wpool.tile([128, 1536], f32, tag="f")
            nc.scalar.activation(f_t[:, :L], l_t[:, :L], Act.Exp, scale=-1.0)
            nc.vector.tensor_mul(OUT[:, c0:c0 + L], num[:, :L], f_t[:, :L])
            # rows fully covered by chunks <= this one stream out immediately
            nc.sync.dma_start(out_d[:, r0:r1, :],
                              OUTv[:, 1 + r0:1 + r1, 1:65])
    nc.finalize()
    return nc


def kernel(x, w_q, w_k, w_v, rel_d, rel_h, rel_w):
    from concourse.bass_utils import run_bass_kernel_spmd

    x = np.asarray(x, np.float32)
    rd = np.asarray(rel_d, np.float32).reshape(21, 3)
    rh = np.asarray(rel_h, np.float32).reshape(21, 3)
    rw = np.asarray(rel_w, np.float32).reshape(22, 3)

    xp = np.zeros((64, 18, HP, HP), np.float32)
    xp[:, 1:17, 1:65, 1:65] = x[0]

    B = np.zeros((128, 27), np.float32)
    for c in range(64):
        for kv in range(27):
            kd, r = divmod(kv, 9)
            kh, kw = divmod(r, 3)
            b = rd[c, kd] if c < 21 else (rh[c - 21, kh] if c < 42 else rw[c - 42, kw])
            B[c, kv] = B[64 + c, kv] = b

    idt = np.eye(128, dtype=np.float32)
    if E_BF16:
        import ml_dtypes
        idt = idt.astype(ml_dtypes.bfloat16)
    com = {
        "wk2": np.concatenate([w_k.T, w_k.T], 1).astype(np.float32).copy(),
        "wv2": np.concatenate([w_v.T, w_v.T], 1).astype(np.float32).copy(),
        "wq2": np.concatenate([w_q.T, w_q.T], 1).astype(np.float32).copy(),
        "bias": B, "ident": idt,
    }
    in_maps = []
    for i in range(8):
        m = dict(com)
        m["xs"] = xp[:, 2 * i:2 * i + 4].reshape(64, NPL * HW).copy()
        in_maps.append(m)

    if "nc" not in _CACHE:
        _CACHE["nc"] = _build()
    res = run_bass_kernel_spmd(_CACHE["nc"], in_maps, list(range(8)))

    out = np.empty((1, 64, 16, 64, 64), np.float32)
    for i in range(8):
        arr = res.results[i]["out"].reshape(2, 64, 64, 64)
        out[0, :, 2 * i] = arr[0]
        out[0, :, 2 * i + 1] = arr[1]
    return out
